# revision 9
# baseline (speedup 1.0000x reference)
"""DescriptorLoss kernel for Trainium2 (8 NeuronCores, SPMD data-parallel).

Math:
    d[b,ij,kl] = sum_c desc0[b,c,ij] * desc1[b,c,kl]
    loss = mean(where(mask, 250*relu(1 - d), relu(d - 0.2)))

Per core: shard = (batch, i-slab) -> 1024 ij rows x 4096 kl cols. The PE
computes d' = 5*d with fp8e4m3 DoubleRow matmuls (contraction 128 packed as
[64 partitions x 2]); the mask enters as u = d' - 2048*m so both hinges live
in disjoint ranges of one value (|d'| < ~1000 << 2048):
    relu(d-0.2)  = relu(u-1)/5          (masked elems die: u-1 < -1000)
    250*relu(1-d) = 50*relu(-u-2043)    (unmasked die; 2044 is fp16-exact)
The positive hinge is folded into the PSUM->SBUF spill via
relu(x) = (x + |x|)/2: sum(u) is computed host-side from the quantized
inputs (exact), so only sum|u-1| is needed on-device - the spill itself
produces it. Per 128x2048 half-group (hg), one of two pipelines:

ACT groups (5): PE also injects -2048*m (diag(-1) @ 2048m, fp8e5m2 DR);
  ACT spills scr = |u-1| fp16 with accum_out = sum|u-1|. One fast DVE/Pool
  pass per group computes the negative hinge:
      accB = sum(max(scr, 2044))  ->  B = accB - 2044*N
DVE groups (3): DVE spills scr' = (psum - 1) - 2048m (scalar_tensor_tensor,
  self-injecting, no PE inject), then two fast all-SBUF fp16 tensor_scalar
  passes (4x DVE perf mode):
      sum|scr'| (abs_max vs 0)  and  accB = sum(min(scr', -2044))
Host: A = (Su - N)/2 + sum|u-1|/2, B as above, loss = sum(A + 250B)/5/count.
"""

import numpy as np
import ml_dtypes

import concourse.bacc as bacc
import concourse.mybir as mybir
import concourse.tile as tile
from concourse.bass_utils import run_bass_kernel_spmd

B, D, H, W = 2, 128, 64, 64
N_CORES = 8
IJ = H * W                 # 4096
ROWS_PER_CORE = IJ // 4    # 1024
G = ROWS_PER_CORE // 128   # 8 row groups of 128
HG_COLS = 2048             # half-group column width
N_HG = G * 2               # 16 half-groups per core
MINJ = 2048.0              # mask offset
THR = 2044.0               # = MINJ - 5 + 1; fp16-exact clamp constant

DVE_GROUPS = (2, 5)                      # spilled+evaluated on DVE
ACT_GROUPS = tuple(g for g in range(G) if g not in DVE_GROUPS)

_cached = {}


def _build_program():
    nc = bacc.Bacc("TRN2")
    f32 = mybir.dt.float32
    f16 = mybir.dt.float16
    f8e4 = mybir.dt.float8e4
    f8e5 = mybir.dt.float8e5
    Alu = mybir.AluOpType
    Act = mybir.ActivationFunctionType
    DR = mybir.MatmulPerfMode.DoubleRow

    a5 = nc.declare_dram_parameter("a5", [64, G, 2, 128], f8e4, isOutput=False)
    bm = nc.declare_dram_parameter("bm", [64, 2, IJ], f8e4, isOutput=False)
    idn = nc.declare_dram_parameter("idn", [64, 2, 128], f8e5, isOutput=False)
    mdr = nc.declare_dram_parameter(
        "mdr", [64, 2 * len(ACT_GROUPS), 2, HG_COLS], f8e5, isOutput=False
    )
    mcons = nc.declare_dram_parameter(
        "mcons", [128, len(DVE_GROUPS), IJ], f8e5, isOutput=False
    )
    accs_out = nc.declare_dram_parameter("accs", [128, 32], f32, isOutput=True)

    with tile.TileContext(nc) as tc:
        with (
            tc.tile_pool(name="desc", bufs=1) as desc_pool,
            tc.tile_pool(name="maskd", bufs=4) as maskd_pool,
            tc.tile_pool(name="maskc", bufs=2) as maskc_pool,
            tc.tile_pool(name="scr", bufs=2) as scr_pool,
            tc.tile_pool(name="gout", bufs=2) as g_pool,
            tc.tile_pool(name="gpout", bufs=2) as gp_pool,
            tc.tile_pool(name="accs", bufs=1) as acc_pool,
            tc.tile_pool(name="psd", bufs=2, space="PSUM") as psum_pool,
        ):
            a5_t = desc_pool.tile([64, G, 2, 128], f8e4, tag="a5")
            bm_t = desc_pool.tile([64, 2, IJ], f8e4, tag="bm")
            id_t = desc_pool.tile([64, 2, 128], f8e5, tag="idn")
            bias_m1 = desc_pool.tile([128, 1], f32, tag="bias")
            nc.sync.dma_start(a5_t[:], a5[:])
            nc.sync.dma_start(bm_t[:], bm[:])
            nc.sync.dma_start(id_t[:], idn[:])
            nc.gpsimd.memset(bias_m1[:], -1.0)

            accT = acc_pool.tile([128, 32], f32, tag="accs")

            na = 0
            scr_t = None
            mc_t = None
            for hg in range(N_HG):
                g, h = hg // 2, hg % 2
                c0 = h * HG_COLS
                on_dve = g in DVE_GROUPS

                if on_dve:
                    if h == 0:
                        mc_t = maskc_pool.tile([128, IJ], f8e5, tag="mc")
                        nc.sync.dma_start(
                            mc_t[:], mcons[:, DVE_GROUPS.index(g), :]
                        )
                else:
                    md_t = maskd_pool.tile([64, 2, HG_COLS], f8e5, tag="md")
                    nc.sync.dma_start(md_t[:], mdr[:, na, :, :])
                    na += 1

                psum_t = psum_pool.tile([128, HG_COLS], f32, tag="d")
                for j in range(HG_COLS // 512):
                    js = slice(j * 512, (j + 1) * 512)
                    cs = slice(c0 + j * 512, c0 + (j + 1) * 512)
                    nc.tensor.matmul(
                        psum_t[:, js], a5_t[:, g, :, :], bm_t[:, :, cs],
                        start=True, stop=on_dve, perf_mode=DR,
                    )
                if not on_dve:
                    for j in range(HG_COLS // 512):
                        js = slice(j * 512, (j + 1) * 512)
                        nc.tensor.matmul(
                            psum_t[:, js], id_t[:], md_t[:, :, js],
                            start=False, stop=True, perf_mode=DR,
                        )

                if h == 0:
                    scr_t = scr_pool.tile([128, IJ], f16, tag="scr")

                if on_dve:
                    nc.vector.scalar_tensor_tensor(
                        scr_t[:, c0:c0 + HG_COLS], psum_t[:], 1.0,
                        mc_t[:, c0:c0 + HG_COLS],
                        op0=Alu.subtract, op1=Alu.subtract,
                    )
                else:
                    nc.scalar.activation(
                        scr_t[:, c0:c0 + HG_COLS], psum_t[:], Act.Abs,
                        bias=bias_m1[:], scale=1.0,
                        accum_out=accT[:, hg:hg + 1],
                    )

                if h == 1:
                    if on_dve:
                        gA = g_pool.tile([128, IJ], f16, tag="g")
                        gB = g_pool.tile([128, IJ], f16, tag="g")
                        nc.vector.tensor_scalar(
                            gA[:], scr_t[:], 0.0, None,
                            op0=Alu.max, op1=Alu.add,
                            accum_out=accT[:, 16 + 2 * g + 1:16 + 2 * g + 2],
                        )
                        nc.vector.tensor_scalar(
                            gB[:], scr_t[:], -THR, None,
                            op0=Alu.min, op1=Alu.add,
                            accum_out=accT[:, 16 + 2 * g:16 + 2 * g + 1],
                        )
                    else:
                        gV = g_pool.tile([128, IJ], f16, tag="g")
                        nc.vector.tensor_scalar(
                            gV[:], scr_t[:], THR, None,
                            op0=Alu.max, op1=Alu.add,
                            accum_out=accT[:, 16 + 2 * g:16 + 2 * g + 1],
                        )

            nc.sync.dma_start(accs_out[:], accT[:])

    nc.finalize()
    return nc


def _prep_inputs(descriptors_0, descriptors_1, similarity_mask):
    d0 = np.asarray(descriptors_0, dtype=np.float32)
    d1 = np.asarray(descriptors_1, dtype=np.float32)
    mkv = np.asarray(similarity_mask)
    in_maps = []
    su_list = []
    for c in range(N_CORES):
        b = c >> 2
        r0 = (c & 3) * ROWS_PER_CORE
        a5 = d0[b].reshape(D, IJ)[:, r0:r0 + ROWS_PER_CORE] * np.float32(5.0)
        a5q8 = a5.astype(ml_dtypes.float8_e4m3fn)
        a5q = a5q8.astype(np.float32)            # [128 chan, 1024 rows]
        # [chan(2,64), row(G,128)] -> [64, G, 2, 128]
        a5dr = np.ascontiguousarray(
            a5q8.reshape(2, 64, G, 128).transpose(1, 2, 0, 3)
        )
        bmq8 = d1[b].reshape(D, IJ).astype(ml_dtypes.float8_e4m3fn)
        bmq = bmq8.astype(np.float32)            # [128 chan, 4096 cols]
        bmdr = np.ascontiguousarray(bmq8.reshape(2, 64, IJ).transpose(1, 0, 2))
        idn = np.zeros((64, 2, 128), dtype=np.float32)
        for t in range(2):
            for p in range(64):
                idn[p, t, t * 64 + p] = -1.0
        idn = idn.astype(ml_dtypes.float8_e5m2)

        mk = mkv[b].reshape(IJ, IJ)[r0:r0 + ROWS_PER_CORE]  # [1024, 4096] bool
        m4k = mk.astype(np.float32) * np.float32(MINJ)
        mdr_list, mcons_list = [], []
        for g in range(G):
            blk = m4k[g * 128:(g + 1) * 128]     # [128, 4096]
            if g in DVE_GROUPS:
                mcons_list.append(blk)
            else:
                # rows [t*64+p] -> [64, 2, cols], split into the 2 hgs
                bdr = blk.reshape(2, 64, IJ).transpose(1, 0, 2)
                mdr_list.append(bdr[:, :, :HG_COLS])
                mdr_list.append(bdr[:, :, HG_COLS:])
        mdr = np.ascontiguousarray(np.stack(mdr_list, axis=1)).astype(
            ml_dtypes.float8_e5m2
        )
        mcons = np.ascontiguousarray(np.stack(mcons_list, axis=1)).astype(
            ml_dtypes.float8_e5m2
        )
        # host-side Su[row] = sum_j (d'_q[row, j]) - 2048 * n_mask[row]
        bsum = bmq.sum(axis=1, dtype=np.float64)
        su = a5q.T.astype(np.float64) @ bsum - float(MINJ) * mk.sum(
            axis=1, dtype=np.float64
        )
        su_list.append(su)                       # [1024]
        in_maps.append(
            {
                "a5": a5dr,
                "bm": bmdr,
                "idn": np.ascontiguousarray(idn),
                "mdr": mdr,
                "mcons": mcons,
            }
        )
    _cached["su"] = su_list
    return in_maps


def _run(in_maps, **kwargs):
    if "nc" not in _cached:
        _cached["nc"] = _build_program()
    return run_bass_kernel_spmd(_cached["nc"], in_maps, list(range(N_CORES)), **kwargs)


def _combine(results):
    su_list = _cached["su"]
    total = 0.0
    for c, r in enumerate(results):
        accs = r["accs"].astype(np.float64)      # [128, 32]
        su = su_list[c].reshape(G, 128)          # [G, 128] per (group, partition)
        # ACT groups: A = (Su - 4096)/2 + sum|u-1|/2 ; DVE groups: A direct
        A = 0.0
        for g in range(G):
            if g in DVE_GROUPS:
                A += accs[:, 16 + 2 * g + 1].sum()
            else:
                sabs = accs[:, 2 * g] + accs[:, 2 * g + 1]
                A += 0.5 * (su[g].sum() - float(IJ) * 128) + 0.5 * sabs.sum()
        # B per group from accB
        Bv = 0.0
        for g in range(G):
            accB = accs[:, 16 + 2 * g].sum()
            if g in DVE_GROUPS:
                Bv += -accB - THR * IJ * 128
            else:
                Bv += accB - THR * IJ * 128
        total += (A + 250.0 * Bv) / 5.0
    return np.float32(total / float(B * IJ * IJ))


def kernel(descriptors_0, descriptors_1, similarity_mask):
    in_maps = _prep_inputs(descriptors_0, descriptors_1, similarity_mask)
    res = _run(in_maps)
    return _combine(res.results)


# revision 10
# speedup vs baseline: 1.0021x; 1.0021x over previous
"""DescriptorLoss kernel for Trainium2 (8 NeuronCores, SPMD data-parallel).

Math:
    d[b,ij,kl] = sum_c desc0[b,c,ij] * desc1[b,c,kl]
    loss = mean(where(mask, 250*relu(1 - d), relu(d - 0.2)))

Per core: shard = (batch, i-slab) -> 1024 ij rows x 4096 kl cols, as 8 row
groups of 128 x 4096, each split into two 2048-col half-groups (hg). The PE
computes d' = 5*d with fp8e4m3 matmuls (same PE rate as bf16, half the
DMA/SBUF; quantization error ~2e-4 << tolerance). Every reduction on TRN2
runs at 1 elem/lane/cycle (accumulating ops have no DVE perf-mode uops), so
the two hinge sums per element are streamed straight from PSUM, split
across both capable engines to balance ACT (1.2 GHz) vs DVE (0.96 GHz):

ACT-hgs (9): PE additionally injects u = d' - 2048*m (diag(-1) @ m8,
  m8 in {0,2048} fp8e5m2) so per-partition-scalar bias suffices:
      accA = sum relu(u - 1)            (masked die: u-1 < -1000)
      accB = sum relu(-u - 2043)        (unmasked die)
DVE-hgs (7): no inject; scalar_tensor_tensor clamps with mask-encoded
  per-element bounds (baseline trick, C = 2048):
      acc1 = sum min(max(d',1), X)   X = 1  if m else  C   -> A = acc1 - N
      acc2 = sum max(min(d',5), Y)   Y = -C if m else  5   -> B = 5N - acc2
Host: loss = sum(A + 250*B)/5 / count.  (A-hinge at d'=1, B-hinge at d'=5,
both exact: accumulators are fp32, no 16-bit intermediates anywhere.)
"""

import numpy as np
import ml_dtypes

import concourse.bacc as bacc
import concourse.mybir as mybir
import concourse.tile as tile
from concourse.bass_utils import run_bass_kernel_spmd

B, D, H, W = 2, 128, 64, 64
N_CORES = 8
IJ = H * W                 # 4096
ROWS_PER_CORE = IJ // 4    # 1024
G = ROWS_PER_CORE // 128   # 8 row groups of 128
HG_COLS = 2048             # half-group column width
N_HG = G * 2               # 16 half-groups per core
C = 2048.0                 # clamp / inject magnitude

DVE_HGS = (1, 3, 5, 7, 9, 11, 13)        # evaluated by DVE STT (no inject)
ACT_HGS = tuple(h for h in range(N_HG) if h not in DVE_HGS)

_cached = {}


def _build_program():
    nc = bacc.Bacc("TRN2")
    f32 = mybir.dt.float32
    bf16 = mybir.dt.bfloat16
    f8e4 = mybir.dt.float8e4
    f8e5 = mybir.dt.float8e5
    Alu = mybir.AluOpType
    Act = mybir.ActivationFunctionType

    a5 = nc.declare_dram_parameter("a5", [D, ROWS_PER_CORE], f8e4, isOutput=False)
    bm = nc.declare_dram_parameter("bm", [D, IJ], f8e4, isOutput=False)
    idn = nc.declare_dram_parameter("idn", [D, D], f8e5, isOutput=False)
    m8 = nc.declare_dram_parameter(
        "m8", [128, len(ACT_HGS), HG_COLS], f8e5, isOutput=False
    )
    x8 = nc.declare_dram_parameter(
        "x8", [128, len(DVE_HGS), HG_COLS], f8e5, isOutput=False
    )
    y8 = nc.declare_dram_parameter(
        "y8", [128, len(DVE_HGS), HG_COLS], f8e5, isOutput=False
    )
    accs_out = nc.declare_dram_parameter("accs", [128, 2 * N_HG], f32, isOutput=True)

    with tile.TileContext(nc) as tc:
        with (
            tc.tile_pool(name="desc", bufs=1) as desc_pool,
            tc.tile_pool(name="mask", bufs=4) as mask_pool,
            tc.tile_pool(name="gout", bufs=4) as g_pool,
            tc.tile_pool(name="accs", bufs=1) as acc_pool,
            tc.tile_pool(name="psd", bufs=2, space="PSUM") as psum_pool,
        ):
            a5_t = desc_pool.tile([D, ROWS_PER_CORE], f8e4, tag="a5")
            bm_t = desc_pool.tile([D, IJ], f8e4, tag="bm")
            id_t = desc_pool.tile([D, D], f8e5, tag="idn")
            bias_a = desc_pool.tile([128, 1], f32, tag="ba")
            bias_b = desc_pool.tile([128, 1], f32, tag="bb")
            nc.sync.dma_start(a5_t[:], a5[:])
            nc.sync.dma_start(bm_t[:], bm[:])
            nc.sync.dma_start(id_t[:], idn[:])
            nc.gpsimd.memset(bias_a[:], -1.0)
            nc.gpsimd.memset(bias_b[:], -(C - 5.0))

            accT = acc_pool.tile([128, 2 * N_HG], f32, tag="accs")

            na = nd = 0
            for hg in range(N_HG):
                g, h = hg // 2, hg % 2
                c0 = h * HG_COLS
                rs = slice(g * 128, (g + 1) * 128)
                on_dve = hg in DVE_HGS

                if on_dve:
                    xm_t = mask_pool.tile([128, HG_COLS], f8e5, tag="x8")
                    ym_t = mask_pool.tile([128, HG_COLS], f8e5, tag="y8")
                    nc.sync.dma_start(xm_t[:], x8[:, nd, :])
                    nc.sync.dma_start(ym_t[:], y8[:, nd, :])
                    nd += 1
                else:
                    mm_t = mask_pool.tile([128, HG_COLS], f8e5, tag="m8")
                    nc.sync.dma_start(mm_t[:], m8[:, na, :])
                    na += 1

                psum_t = psum_pool.tile([128, HG_COLS], f32, tag="d")
                for j in range(HG_COLS // 512):
                    js = slice(j * 512, (j + 1) * 512)
                    cs = slice(c0 + j * 512, c0 + (j + 1) * 512)
                    nc.tensor.matmul(
                        psum_t[:, js], a5_t[:, rs], bm_t[:, cs],
                        start=True, stop=on_dve,
                    )
                if not on_dve:
                    for j in range(HG_COLS // 512):
                        js = slice(j * 512, (j + 1) * 512)
                        nc.tensor.matmul(
                            psum_t[:, js], id_t[:], mm_t[:, js],
                            start=False, stop=True,
                        )

                if on_dve:
                    g1 = g_pool.tile([128, HG_COLS], bf16, tag="g")
                    g2 = g_pool.tile([128, HG_COLS], bf16, tag="g")
                    nc.vector.scalar_tensor_tensor(
                        g1[:], psum_t[:], 1.0, xm_t[:],
                        op0=Alu.max, op1=Alu.min,
                        accum_out=accT[:, 2 * hg:2 * hg + 1],
                    )
                    nc.vector.scalar_tensor_tensor(
                        g2[:], psum_t[:], 5.0, ym_t[:],
                        op0=Alu.min, op1=Alu.max,
                        accum_out=accT[:, 2 * hg + 1:2 * hg + 2],
                    )
                else:
                    g1 = g_pool.tile([128, HG_COLS], bf16, tag="g")
                    g2 = g_pool.tile([128, HG_COLS], bf16, tag="g")
                    nc.scalar.activation(
                        g1[:], psum_t[:], Act.Relu,
                        bias=bias_a[:], scale=1.0,
                        accum_out=accT[:, 2 * hg:2 * hg + 1],
                    )
                    nc.scalar.activation(
                        g2[:], psum_t[:], Act.Relu,
                        bias=bias_b[:], scale=-1.0,
                        accum_out=accT[:, 2 * hg + 1:2 * hg + 2],
                    )

            nc.sync.dma_start(accs_out[:], accT[:])

    nc.finalize()
    return nc


def _prep_inputs(descriptors_0, descriptors_1, similarity_mask):
    d0 = np.asarray(descriptors_0, dtype=np.float32)
    d1 = np.asarray(descriptors_1, dtype=np.float32)
    mkv = np.asarray(similarity_mask)
    Cf = np.float32(C)
    idn = np.zeros((D, D), dtype=np.float32)
    np.fill_diagonal(idn, -1.0)
    idn = np.ascontiguousarray(idn.astype(ml_dtypes.float8_e5m2))
    in_maps = []
    for c in range(N_CORES):
        b = c >> 2
        r0 = (c & 3) * ROWS_PER_CORE
        a5 = (d0[b].reshape(D, IJ)[:, r0:r0 + ROWS_PER_CORE] * np.float32(5.0))
        a5q = np.ascontiguousarray(a5.astype(ml_dtypes.float8_e4m3fn))
        bmq = np.ascontiguousarray(
            d1[b].reshape(D, IJ).astype(ml_dtypes.float8_e4m3fn)
        )
        mk = mkv[b].reshape(IJ, IJ)[r0:r0 + ROWS_PER_CORE]  # [1024, 4096] bool
        m8l, x8l, y8l = [], [], []
        for hg in range(N_HG):
            g, h = hg // 2, hg % 2
            blk = mk[g * 128:(g + 1) * 128, h * HG_COLS:(h + 1) * HG_COLS]
            if hg in DVE_HGS:
                x8l.append(np.where(blk, np.float32(1.0), Cf))
                y8l.append(np.where(blk, -Cf, np.float32(5.0)))
            else:
                m8l.append(blk.astype(np.float32) * Cf)
        in_maps.append(
            {
                "a5": a5q,
                "bm": bmq,
                "idn": idn,
                "m8": np.ascontiguousarray(np.stack(m8l, axis=1)).astype(
                    ml_dtypes.float8_e5m2
                ),
                "x8": np.ascontiguousarray(np.stack(x8l, axis=1)).astype(
                    ml_dtypes.float8_e5m2
                ),
                "y8": np.ascontiguousarray(np.stack(y8l, axis=1)).astype(
                    ml_dtypes.float8_e5m2
                ),
            }
        )
    return in_maps


def _run(in_maps, **kwargs):
    if "nc" not in _cached:
        _cached["nc"] = _build_program()
    return run_bass_kernel_spmd(_cached["nc"], in_maps, list(range(N_CORES)), **kwargs)


def _combine(results):
    total = 0.0
    n_hg = 128 * HG_COLS
    for r in results:
        accs = r["accs"].astype(np.float64)      # [128, 32]
        for hg in range(N_HG):
            acc1 = accs[:, 2 * hg].sum()
            acc2 = accs[:, 2 * hg + 1].sum()
            if hg in DVE_HGS:
                A = acc1 - n_hg
                Bv = 5.0 * n_hg - acc2
            else:
                A = acc1
                Bv = acc2
            total += (A + 250.0 * Bv) / 5.0
    return np.float32(total / float(B * IJ * IJ))


def kernel(descriptors_0, descriptors_1, similarity_mask):
    in_maps = _prep_inputs(descriptors_0, descriptors_1, similarity_mask)
    res = _run(in_maps)
    return _combine(res.results)


# revision 12
# speedup vs baseline: 1.0777x; 1.0754x over previous
"""DescriptorLoss kernel for Trainium2 (8 NeuronCores, SPMD data-parallel).

Math:
    d[b,ij,kl] = sum_c desc0[b,c,ij] * desc1[b,c,kl]
    loss = mean(where(mask, 250*relu(1 - d), relu(d - 0.2)))

Per core: shard = (batch, i-slab) -> 1024 ij rows x 4096 kl cols, as 8 row
groups of 128 x 4096, each split into two 2048-col half-groups (hg = one
4-bank PSUM tile). The PE computes d' = 5*d with fp8e4m3 matmuls (same PE
rate as bf16, half the DMA/SBUF; quantization error ~2e-4 << tolerance) and
injects the mask with one extra matmul per 1024 cols:
    u = d' - 2048*m        (diag(-1) @ m8, m8 in {0, 2048} fp8e5m2)
With |d'| < ~1100 << 2048 the two hinges live in disjoint ranges of u, so
each needs only scalar constants. Every accumulating op on TRN2 runs at
1 elem/lane/cycle (no perf-mode uops for reductions), so the two hinge
sums stream straight from PSUM CONCURRENTLY on the two capable engines:
    ACT:  accA = sum relu(u - 1)        -> A = accA        (masked die)
    DVE:  accB = sum min(u, -2043)      -> B = -accB - 2043*N  (unmasked
                                            clamp; masked give relu(5-d'))
Host: loss = sum(A + 250*B)/5 / count. Accumulators are fp32 - the only
approximation anywhere is the fp8e4m3 descriptor quantization.
"""

import numpy as np
import ml_dtypes

import concourse.bacc as bacc
import concourse.mybir as mybir
import concourse.tile as tile
from concourse.bass_utils import run_bass_kernel_spmd

B, D, H, W = 2, 128, 64, 64
N_CORES = 8
IJ = H * W                 # 4096
ROWS_PER_CORE = IJ // 4    # 1024
G = ROWS_PER_CORE // 128   # 8 row groups of 128
HG_COLS = 2048             # half-group column width (one 4-bank PSUM tile)
N_HG = G * 2               # 16 half-groups per core
C = 2048.0                 # mask inject magnitude
THR = C - 5.0              # 2043: B-hinge clamp constant

MMF = 512                  # matmul moving free dim (cols per matmul)

_cached = {}


def _build_program():
    nc = bacc.Bacc("TRN2")
    f32 = mybir.dt.float32
    bf16 = mybir.dt.bfloat16
    f8e4 = mybir.dt.float8e4
    f8e5 = mybir.dt.float8e5
    Alu = mybir.AluOpType
    Act = mybir.ActivationFunctionType

    a5 = nc.declare_dram_parameter("a5", [D, ROWS_PER_CORE], f8e4, isOutput=False)
    bm = nc.declare_dram_parameter("bm", [D, IJ], f8e4, isOutput=False)
    idn = nc.declare_dram_parameter("idn", [D, D], f8e5, isOutput=False)
    m8 = nc.declare_dram_parameter("m8", [128, N_HG, HG_COLS], f8e5, isOutput=False)
    accs_out = nc.declare_dram_parameter("accs", [128, 2 * N_HG], f32, isOutput=True)

    with tile.TileContext(nc) as tc:
        with (
            tc.tile_pool(name="desc", bufs=1) as desc_pool,
            tc.tile_pool(name="mask", bufs=4) as mask_pool,
            tc.tile_pool(name="gout", bufs=4) as g_pool,
            tc.tile_pool(name="accs", bufs=1) as acc_pool,
            tc.tile_pool(name="psd", bufs=2, space="PSUM") as psum_pool,
        ):
            a5_t = desc_pool.tile([D, ROWS_PER_CORE], f8e4, tag="a5")
            bm_t = desc_pool.tile([D, IJ], f8e4, tag="bm")
            id_t = desc_pool.tile([D, D], f8e5, tag="idn")
            bias_a = desc_pool.tile([128, 1], f32, tag="ba")
            nc.sync.dma_start(a5_t[:], a5[:])
            nc.sync.dma_start(bm_t[:], bm[:])
            nc.sync.dma_start(id_t[:], idn[:])
            nc.gpsimd.memset(bias_a[:], -1.0)

            accT = acc_pool.tile([128, 2 * N_HG], f32, tag="accs")

            for hg in range(N_HG):
                g, h = hg // 2, hg % 2
                c0 = h * HG_COLS
                rs = slice(g * 128, (g + 1) * 128)

                mm_t = mask_pool.tile([128, HG_COLS], f8e5, tag="m8")
                nc.sync.dma_start(mm_t[:], m8[:, hg, :])

                psum_t = psum_pool.tile([128, HG_COLS], f32, tag="d")
                for j in range(HG_COLS // MMF):
                    js = slice(j * MMF, (j + 1) * MMF)
                    cs = slice(c0 + j * MMF, c0 + (j + 1) * MMF)
                    nc.tensor.matmul(
                        psum_t[:, js], a5_t[:, rs], bm_t[:, cs],
                        start=True, stop=False,
                    )
                for j in range(HG_COLS // MMF):
                    js = slice(j * MMF, (j + 1) * MMF)
                    nc.tensor.matmul(
                        psum_t[:, js], id_t[:], mm_t[:, js],
                        start=False, stop=True,
                    )

                g1 = g_pool.tile([128, HG_COLS], bf16, tag="g")
                g2 = g_pool.tile([128, HG_COLS], bf16, tag="g")
                nc.scalar.activation(
                    g1[:], psum_t[:], Act.Relu,
                    bias=bias_a[:], scale=1.0,
                    accum_out=accT[:, 2 * hg:2 * hg + 1],
                )
                nc.vector.tensor_scalar(
                    g2[:], psum_t[:], -THR, None,
                    op0=Alu.min, op1=Alu.add,
                    accum_out=accT[:, 2 * hg + 1:2 * hg + 2],
                )

            nc.sync.dma_start(accs_out[:], accT[:])

    nc.finalize()
    return nc


def _prep_inputs(descriptors_0, descriptors_1, similarity_mask):
    d0 = np.asarray(descriptors_0, dtype=np.float32)
    d1 = np.asarray(descriptors_1, dtype=np.float32)
    mkv = np.asarray(similarity_mask)
    idn = np.zeros((D, D), dtype=np.float32)
    np.fill_diagonal(idn, -1.0)
    idn = np.ascontiguousarray(idn.astype(ml_dtypes.float8_e5m2))
    in_maps = []
    for c in range(N_CORES):
        b = c >> 2
        r0 = (c & 3) * ROWS_PER_CORE
        a5 = (d0[b].reshape(D, IJ)[:, r0:r0 + ROWS_PER_CORE] * np.float32(5.0))
        a5q = np.ascontiguousarray(a5.astype(ml_dtypes.float8_e4m3fn))
        bmq = np.ascontiguousarray(
            d1[b].reshape(D, IJ).astype(ml_dtypes.float8_e4m3fn)
        )
        mk = mkv[b].reshape(IJ, IJ)[r0:r0 + ROWS_PER_CORE]  # [1024, 4096] bool
        # [row(G,128), col(2,2048)] -> [128, hg=(G,2), 2048]
        m4k = (mk.astype(np.float32) * np.float32(C)).reshape(G, 128, 2, HG_COLS)
        m8 = np.ascontiguousarray(m4k.transpose(1, 0, 2, 3).reshape(
            128, N_HG, HG_COLS
        )).astype(ml_dtypes.float8_e5m2)
        in_maps.append({"a5": a5q, "bm": bmq, "idn": idn, "m8": m8})
    return in_maps


def _run(in_maps, **kwargs):
    if "nc" not in _cached:
        _cached["nc"] = _build_program()
    return run_bass_kernel_spmd(_cached["nc"], in_maps, list(range(N_CORES)), **kwargs)


def _combine(results):
    total = 0.0
    n_hg = 128 * HG_COLS
    for r in results:
        accs = r["accs"].astype(np.float64)      # [128, 2*N_HG]
        A = accs[:, 0::2].sum()
        Bv = -accs[:, 1::2].sum() - THR * n_hg * N_HG
        total += (A + 250.0 * Bv) / 5.0
    return np.float32(total / float(B * IJ * IJ))


def kernel(descriptors_0, descriptors_1, similarity_mask):
    in_maps = _prep_inputs(descriptors_0, descriptors_1, similarity_mask)
    res = _run(in_maps)
    return _combine(res.results)


# revision 13
# speedup vs baseline: 1.1039x; 1.0243x over previous
"""DescriptorLoss kernel for Trainium2 (8 NeuronCores, SPMD data-parallel).

Math:
    d[b,ij,kl] = sum_c desc0[b,c,ij] * desc1[b,c,kl]
    loss = mean(where(mask, 250*relu(1 - d), relu(d - 0.2)))

Per core: shard = (batch, i-slab) -> 1024 ij rows x 4096 kl cols, as 8 row
groups of 128 x 4096, each split into two 2048-col half-groups (hg = one
4-bank PSUM tile). The PE computes d' = 5*d with fp8e4m3 matmuls (same PE
rate as bf16, half the DMA/SBUF; quantization error ~2e-4 << tolerance) and
injects the mask with one extra matmul per 1024 cols:
    u = d' - 2048*m        (diag(-1) @ m8, m8 in {0, 2048} fp8e5m2)
With |d'| < ~1100 << 2048 the two hinges live in disjoint ranges of u, so
each needs only scalar constants. Every accumulating op on TRN2 runs at
1 elem/lane/cycle (no perf-mode uops for reductions), so the two hinge
sums stream straight from PSUM CONCURRENTLY on the two capable engines:
    ACT:  accA = sum relu(u - 1)        -> A = accA        (masked die)
    DVE:  accB = sum min(u, -2043)      -> B = -accB - 2043*N  (unmasked
                                            clamp; masked give relu(5-d'))
Host: loss = sum(A + 250*B)/5 / count. Accumulators are fp32 - the only
approximation anywhere is the fp8e4m3 descriptor quantization.
"""

import numpy as np
import ml_dtypes

import concourse.bacc as bacc
import concourse.mybir as mybir
import concourse.tile as tile
from concourse.bass_utils import run_bass_kernel_spmd

B, D, H, W = 2, 128, 64, 64
N_CORES = 8
IJ = H * W                 # 4096
ROWS_PER_CORE = IJ // 4    # 1024
G = ROWS_PER_CORE // 128   # 8 row groups of 128
HG_COLS = 2048             # half-group column width (one 4-bank PSUM tile)
N_HG = G * 2               # 16 half-groups per core
C = 2048.0                 # mask inject magnitude
THR = C - 5.0              # 2043: B-hinge clamp constant

MMF = 512                  # matmul moving free dim (cols per matmul)

_cached = {}


def _build_program():
    nc = bacc.Bacc("TRN2")
    f32 = mybir.dt.float32
    bf16 = mybir.dt.bfloat16
    f8e4 = mybir.dt.float8e4
    f8e5 = mybir.dt.float8e5
    Alu = mybir.AluOpType
    Act = mybir.ActivationFunctionType

    a5 = nc.declare_dram_parameter("a5", [D, ROWS_PER_CORE], f8e4, isOutput=False)
    bm = nc.declare_dram_parameter("bm", [D, IJ], f8e4, isOutput=False)
    idn = nc.declare_dram_parameter("idn", [D, D], f8e5, isOutput=False)
    m8 = nc.declare_dram_parameter("m8", [128, N_HG, HG_COLS], f8e5, isOutput=False)
    accs_out = nc.declare_dram_parameter("accs", [128, 2 * N_HG], f32, isOutput=True)

    with tile.TileContext(nc) as tc:
        with (
            tc.tile_pool(name="desc", bufs=1) as desc_pool,
            tc.tile_pool(name="mask", bufs=4) as mask_pool,
            tc.tile_pool(name="gout", bufs=4) as g_pool,
            tc.tile_pool(name="accs", bufs=1) as acc_pool,
            tc.tile_pool(name="psd", bufs=2, space="PSUM") as psum_pool,
        ):
            a5_t = desc_pool.tile([D, ROWS_PER_CORE], f8e4, tag="a5")
            bm_t = desc_pool.tile([D, IJ], f8e4, tag="bm")
            id_t = desc_pool.tile([D, D], f8e5, tag="idn")
            bias_a = desc_pool.tile([128, 1], f32, tag="ba")
            nc.sync.dma_start(a5_t[:], a5[:])
            nc.sync.dma_start(bm_t[:], bm[:])
            nc.sync.dma_start(id_t[:], idn[:])
            nc.gpsimd.memset(bias_a[:], -1.0)

            accA_t = acc_pool.tile([128, N_HG], f32, tag="accsA")
            accB_t = acc_pool.tile([128, N_HG], f32, tag="accsB")

            for hg in range(N_HG):
                g, h = hg // 2, hg % 2
                c0 = h * HG_COLS
                rs = slice(g * 128, (g + 1) * 128)

                mm_t = mask_pool.tile([128, HG_COLS], f8e5, tag="m8")
                nc.sync.dma_start(mm_t[:], m8[:, hg, :])

                psum_t = psum_pool.tile([128, HG_COLS], f32, tag="d")
                for j in range(HG_COLS // MMF):
                    js = slice(j * MMF, (j + 1) * MMF)
                    cs = slice(c0 + j * MMF, c0 + (j + 1) * MMF)
                    nc.tensor.matmul(
                        psum_t[:, js], a5_t[:, rs], bm_t[:, cs],
                        start=True, stop=False,
                    )
                for j in range(HG_COLS // MMF):
                    js = slice(j * MMF, (j + 1) * MMF)
                    nc.tensor.matmul(
                        psum_t[:, js], id_t[:], mm_t[:, js],
                        start=False, stop=True,
                    )

                g1 = g_pool.tile([128, HG_COLS], bf16, tag="g")
                g2 = g_pool.tile([128, HG_COLS], bf16, tag="g")
                nc.scalar.activation(
                    g1[:], psum_t[:], Act.Relu,
                    bias=bias_a[:], scale=1.0,
                    accum_out=accA_t[:, hg:hg + 1],
                )
                nc.vector.tensor_scalar(
                    g2[:], psum_t[:], -THR, None,
                    op0=Alu.min, op1=Alu.add,
                    accum_out=accB_t[:, hg:hg + 1],
                )

            nc.sync.dma_start(accs_out[:, :N_HG], accA_t[:])
            nc.sync.dma_start(accs_out[:, N_HG:], accB_t[:])

    nc.finalize()
    return nc


def _prep_inputs(descriptors_0, descriptors_1, similarity_mask):
    d0 = np.asarray(descriptors_0, dtype=np.float32)
    d1 = np.asarray(descriptors_1, dtype=np.float32)
    mkv = np.asarray(similarity_mask)
    idn = np.zeros((D, D), dtype=np.float32)
    np.fill_diagonal(idn, -1.0)
    idn = np.ascontiguousarray(idn.astype(ml_dtypes.float8_e5m2))
    in_maps = []
    for c in range(N_CORES):
        b = c >> 2
        r0 = (c & 3) * ROWS_PER_CORE
        a5 = (d0[b].reshape(D, IJ)[:, r0:r0 + ROWS_PER_CORE] * np.float32(5.0))
        a5q = np.ascontiguousarray(a5.astype(ml_dtypes.float8_e4m3fn))
        bmq = np.ascontiguousarray(
            d1[b].reshape(D, IJ).astype(ml_dtypes.float8_e4m3fn)
        )
        mk = mkv[b].reshape(IJ, IJ)[r0:r0 + ROWS_PER_CORE]  # [1024, 4096] bool
        # [row(G,128), col(2,2048)] -> [128, hg=(G,2), 2048]
        m4k = (mk.astype(np.float32) * np.float32(C)).reshape(G, 128, 2, HG_COLS)
        m8 = np.ascontiguousarray(m4k.transpose(1, 0, 2, 3).reshape(
            128, N_HG, HG_COLS
        )).astype(ml_dtypes.float8_e5m2)
        in_maps.append({"a5": a5q, "bm": bmq, "idn": idn, "m8": m8})
    return in_maps


def _run(in_maps, **kwargs):
    if "nc" not in _cached:
        _cached["nc"] = _build_program()
    return run_bass_kernel_spmd(_cached["nc"], in_maps, list(range(N_CORES)), **kwargs)


def _combine(results):
    total = 0.0
    n_hg = 128 * HG_COLS
    for r in results:
        accs = r["accs"].astype(np.float64)      # [128, 2*N_HG]
        A = accs[:, :N_HG].sum()
        Bv = -accs[:, N_HG:].sum() - THR * n_hg * N_HG
        total += (A + 250.0 * Bv) / 5.0
    return np.float32(total / float(B * IJ * IJ))


def kernel(descriptors_0, descriptors_1, similarity_mask):
    in_maps = _prep_inputs(descriptors_0, descriptors_1, similarity_mask)
    res = _run(in_maps)
    return _combine(res.results)
